# revision 3
# baseline (speedup 1.0000x reference)
"""Multi-head causal attention (B=2, S=2048, D=1024, H=16) on 8 trn2 NeuronCores.

Strategy (tensor-parallel over heads, hint-compliant):
  - Each core owns 2 heads (128 of 1024 hidden dims): W_q/W_k/W_v column-parallel.
  - Activations kept transposed ([dim, token]) end to end so every matmul
    contracts on the partition axis with zero on-device transposes of x.
  - scores^T = K^T.T @ Q^T per 128-key-chunk x 512-query-tile, two heads packed
    into disjoint PE row-groups (contraction is only dk=64).
  - softmax without max-subtraction (scores are O(1)); rowsum folded into the
    PV matmul via an augmented V [keys, 64+1] whose last column is ones.
  - ctx re-sharded token-parallel with one AllToAll, then the output projection
    runs with full W_o on each core for its 512 tokens.
  - All matmul inputs are float32r (4-byte fp32 data, fast PE mode).

kernel(**inputs) takes the full unsharded inputs and returns the full output.
"""

import numpy as np

import concourse.bass as bass
import concourse.mybir as mybir
import concourse.tile as tile
from concourse import bacc
from concourse.bass import ts
from concourse.bass_utils import run_bass_kernel_spmd

B, S, D = 2, 2048, 1024
H, DK = 16, 64
NCORE = 8
T = B * S          # 4096 tokens
TT = 512           # token tile (projections, q-tiles, out-proj)
NT = T // TT       # 8
KC = 128           # key chunk
QT_PER_B = S // TT   # 4 q-tiles per batch
SCALE = 1.0 / np.sqrt(DK)

f32 = mybir.dt.float32
f32r = mybir.dt.float32r
EXP = mybir.ActivationFunctionType.Exp
MULT = mybir.AluOpType.mult


def build_program():
    nc = bacc.Bacc("TRN2", target_bir_lowering=False, debug=False,
                   num_devices=NCORE)

    xT_d = nc.dram_tensor("xT", [128, 8, T], f32r, kind="ExternalInput").ap()
    wT_d = nc.dram_tensor("wT", [128, 8, 3, 128], f32r, kind="ExternalInput").ap()
    woT_d = nc.dram_tensor("woT", [8, 128, 8, 128], f32r, kind="ExternalInput").ap()
    bqkv_d = nc.dram_tensor("bqkv", [128, 3], f32, kind="ExternalInput").ap()
    bo_d = nc.dram_tensor("bo", [128, 8], f32, kind="ExternalInput").ap()
    masks_d = nc.dram_tensor("masks", [128, 4, 2 * TT], f32r, kind="ExternalInput").ap()
    ones_d = nc.dram_tensor("ones", [1, DK], f32r, kind="ExternalInput").ap()
    onescol_d = nc.dram_tensor("onescol", [128, S // KC], f32r, kind="ExternalInput").ap()
    ident_d = nc.dram_tensor("ident", [128, 128], f32r, kind="ExternalInput").ap()
    outT_d = nc.dram_tensor("outT", [8, 128, TT], f32, kind="ExternalOutput").ap()

    with tile.TileContext(nc) as tc:
        with (
            tc.tile_pool(name="const", bufs=1) as constp,
            tc.tile_pool(name="wostream", bufs=3) as wop,
            tc.tile_pool(name="xstream", bufs=2) as xp,
            tc.tile_pool(name="qkv", bufs=1) as qkvp,
            tc.tile_pool(name="vaug", bufs=1) as vaugp,
            tc.tile_pool(name="ptile", bufs=3) as pp,
            tc.tile_pool(name="small", bufs=2) as smallp,
            tc.tile_pool(name="outsb", bufs=2) as outp,
            tc.tile_pool(name="ps_s", bufs=2, space="PSUM") as ps_s,
            tc.tile_pool(name="ps_ctx", bufs=1, space="PSUM") as ps_ctx,
            tc.tile_pool(name="ps_misc", bufs=2, space="PSUM") as ps_misc,
            tc.tile_pool(name="dram", bufs=1, space="DRAM") as dramp,
        ):
            # ---- constants ----
            wT = constp.tile([128, 8, 3, 128], f32r, tag="wT")
            nc.sync.dma_start(wT[:], wT_d)
            bqkv = constp.tile([128, 3], f32, tag="bqkv")
            nc.sync.dma_start(bqkv[:], bqkv_d)
            bo_sb = constp.tile([128, 8], f32, tag="bo")
            nc.sync.dma_start(bo_sb[:], bo_d)
            masks = constp.tile([128, 4, 2 * TT], f32r, tag="masks")
            nc.sync.dma_start(masks[:], masks_d)
            ones1 = constp.tile([1, DK], f32r, tag="ones")
            nc.sync.dma_start(ones1[:], ones_d)
            ident = constp.tile([128, 128], f32r, tag="ident")
            nc.sync.dma_start(ident[:], ident_d)

            # ---- stage B: QKV projections (transposed activations) ----
            qkv = [qkvp.tile([128, T], f32r, tag=f"qkv{j}", name=f"qkv{j}") for j in range(3)]
            for t in range(NT):
                xt = xp.tile([128, 8, TT], f32r, tag="xt")
                nc.sync.dma_start(xt[:], xT_d[:, :, ts(t, TT)])
                for j in range(3):
                    ps = ps_misc.tile([128, TT], f32, tag="mm")
                    for o in range(8):
                        nc.tensor.matmul(ps[:], wT[:, o, j, :], xt[:, o, :],
                                         start=(o == 0), stop=(o == 7))
                    nc.vector.tensor_scalar_add(qkv[j][:, ts(t, TT)], ps[:],
                                                bqkv[:, j:j + 1])

            qT_sb, kT_sb, vT_sb = qkv

            # ---- stage C: V^T -> natural-layout V_aug [keys, 65] per (b, head) ----
            vaug = [[vaugp.tile([128, S // KC, DK + 1], f32r, tag=f"vaug{b}{h}", name=f"vaug{b}{h}")
                     for h in range(2)] for b in range(B)]
            for b in range(B):
                for h in range(2):
                    nc.sync.dma_start(vaug[b][h][:, :, DK:DK + 1],
                                      onescol_d[:, :, None])
                for kt in range(S // KC):
                    ps_t = ps_misc.tile([128, TT], f32r, tag="mm")
                    nc.tensor.transpose(ps_t[:, 0:128],
                                        vT_sb[:, b * S + kt * KC:b * S + (kt + 1) * KC],
                                        ident[:])
                    for h in range(2):
                        nc.vector.tensor_copy(vaug[b][h][:, kt, 0:DK],
                                              ps_t[:, DK * h:DK * h + DK])

            # ---- stage D: attention per (batch, q-tile of 512) ----
            a2a_in = dramp.tile([NT, 128, TT], f32r, tag="a2a_in")
            a2a_out = dramp.tile([NT, 128, TT], f32r, tag="a2a_out")

            for b in range(B):
                for j in range(QT_PER_B):
                    nk = 4 * (j + 1)            # key chunks of 128
                    qs = b * S + j * TT
                    pc = [ps_ctx.tile([DK + 1, TT], f32, tag=f"c{h}", name=f"pc{h}")
                          for h in range(2)]

                    def emit_pv(p_tile, m, nk=nk, pc=pc, b=b):
                        for h in range(2):
                            nc.tensor.matmul(
                                pc[h][:], vaug[b][h][:, m, :],
                                p_tile[:, TT * h:TT * (h + 1)],
                                start=(m == 0), stop=(m == nk - 1),
                                skip_group_check=True)

                    prev = None
                    for m in range(nk):
                        ks = b * S + m * KC
                        ps = ps_s.tile([128, 2 * TT], f32, tag="s")
                        nc.tensor.matmul(ps[:, 0:TT], kT_sb[0:DK, ks:ks + KC],
                                         qT_sb[0:DK, qs:qs + TT],
                                         start=True, stop=True, tile_position=(0, 0))
                        nc.tensor.matmul(ps[:, TT:], kT_sb[DK:128, ks:ks + KC],
                                         qT_sb[DK:128, qs:qs + TT],
                                         start=True, stop=True, tile_position=(64, 0))
                        p = pp.tile([128, 2 * TT], f32r, tag="p")
                        nc.scalar.activation(p[:], ps[:], EXP, scale=float(SCALE))
                        if m >= 4 * j:
                            nc.vector.tensor_tensor(p[:], p[:],
                                                    masks[:, m - 4 * j, :], MULT)
                        if prev is not None:
                            emit_pv(*prev)
                        prev = (p, m)
                    emit_pv(*prev)

                    # normalize by the fused rowsum (row DK of pc) and ship out
                    blk = b * QT_PER_B + j
                    for h in range(2):
                        rc = smallp.tile([1, TT], f32r, tag="rc")
                        with nc.allow_low_precision(reason="softmax denominator"):
                            nc.vector.reciprocal(rc[:], pc[h][DK:DK + 1, :])
                        psb = ps_misc.tile([128, TT], f32, tag="mm")
                        nc.tensor.matmul(psb[0:DK, :], ones1[:], rc[:],
                                         start=True, stop=True)
                        cx = smallp.tile([DK, TT], f32, tag="cx")
                        nc.vector.tensor_copy(cx[:], pc[h][0:DK, :])
                        cxn = smallp.tile([DK, TT], f32r, tag="cxn")
                        nc.vector.tensor_tensor(cxn[:], cx[:], psb[0:DK, :], MULT)
                        nc.sync.dma_start(a2a_in[blk, DK * h:DK * (h + 1), :], cxn[:])

            # ---- A2A: head-sharded ctx -> token-sharded full ctx ----
            nc.gpsimd.collective_compute(
                "AllToAll", mybir.AluOpType.bypass,
                replica_groups=[list(range(NCORE))],
                ins=[a2a_in[:].opt()], outs=[a2a_out[:].opt()])

            # ---- stage E: out-projection for this core's 512 tokens ----
            ctx_sb = constp.tile([128, 8, TT], f32r, tag="ctx")
            nc.sync.dma_start(ctx_sb[:], a2a_out[:].rearrange("r p t -> p r t"))
            for o in range(8):
                wo_t = wop.tile([128, 8, 128], f32r, tag="wo")
                nc.sync.dma_start(wo_t[:], woT_d[o])
                ps = ps_misc.tile([128, TT], f32, tag="mm")
                for d in range(8):
                    nc.tensor.matmul(ps[:], wo_t[:, d, :], ctx_sb[:, d, :],
                                     start=(d == 0), stop=(d == 7))
                ot = outp.tile([128, TT], f32, tag="ot")
                nc.vector.tensor_scalar_add(ot[:], ps[:], bo_sb[:, o:o + 1])
                nc.sync.dma_start(outT_d[o], ot[:])

    nc.compile()
    return nc


def make_in_maps(x, Wq, bq, Wk, bk, Wv, bv, Wo, bo):
    x = np.asarray(x, np.float32)
    xT = np.ascontiguousarray(x.reshape(T, D).T)                  # [D, T]
    xT_t = np.ascontiguousarray(xT.reshape(8, 128, T).transpose(1, 0, 2))

    woT = np.ascontiguousarray(
        np.asarray(Wo, np.float32).T.reshape(8, 128, 8, 128).transpose(2, 1, 0, 3))

    mask_half = (np.arange(128)[:, None, None] + 128 * np.arange(4)[None, :, None]
                 <= np.arange(TT)[None, None, :]).astype(np.float32)  # [128,4,512]
    masks = np.ascontiguousarray(np.concatenate([mask_half, mask_half], axis=2))

    ones = np.ones((1, DK), np.float32)
    onescol = np.ones((128, S // KC), np.float32)
    ident = np.eye(128, dtype=np.float32)
    bo_t = np.ascontiguousarray(np.asarray(bo, np.float32).reshape(8, 128).T)

    in_maps = []
    for c in range(NCORE):
        sl = slice(128 * c, 128 * (c + 1))
        wT_c = np.stack(
            [np.ascontiguousarray(
                np.asarray(W, np.float32)[sl, :].T.reshape(8, 128, 128)
                .transpose(1, 0, 2))
             for W in (Wq, Wk, Wv)], axis=2)                       # [128, 8, 3, 128]
        bqkv_c = np.stack([np.asarray(b_, np.float32)[sl]
                           for b_ in (bq, bk, bv)], axis=1)        # [128, 3]
        in_maps.append({
            "xT": xT_t,
            "wT": np.ascontiguousarray(wT_c),
            "woT": woT,
            "bqkv": np.ascontiguousarray(bqkv_c),
            "bo": bo_t,
            "masks": masks,
            "ones": ones,
            "onescol": onescol,
            "ident": ident,
        })
    return in_maps


def assemble_output(results):
    outT = np.empty((D, T), np.float32)
    for c in range(NCORE):
        outT[:, TT * c:TT * (c + 1)] = results[c]["outT"].reshape(D, TT)
    return np.ascontiguousarray(outT.T).reshape(B, S, D)


_PROGRAM = None


def get_program():
    global _PROGRAM
    if _PROGRAM is None:
        _PROGRAM = build_program()
    return _PROGRAM


def run(in_maps, **kwargs):
    nc = get_program()
    return run_bass_kernel_spmd(nc, in_maps, core_ids=list(range(NCORE)), **kwargs)


def kernel(x, Wq, bq, Wk, bk, Wv, bv, Wo, bo):
    in_maps = make_in_maps(x, Wq, bq, Wk, bk, Wv, bv, Wo, bo)
    res = run(in_maps)
    return assemble_output(res.results)


if __name__ == "__main__":
    rng = np.random.default_rng(0)
    x = rng.standard_normal((B, S, D), dtype=np.float32)
    mk = lambda *s: (rng.random(s, np.float32) - 0.5) / 16
    out = kernel(x, mk(D, D), mk(D), mk(D, D), mk(D), mk(D, D), mk(D),
                 mk(D, D), mk(D))
    print(out.shape, out.dtype, np.abs(out).mean())


# revision 8
# speedup vs baseline: 1.5460x; 1.5460x over previous
"""Multi-head causal attention (B=2, S=2048, D=1024, H=16) on 8 trn2 NeuronCores.

Strategy (tensor-parallel over heads, per the sharding hint):
  - Each core owns 2 heads (128 of 1024 hidden dims): W_q/W_k/W_v column-parallel.
  - Activations kept transposed ([dim, token]) end to end so every matmul
    contracts on the partition axis with zero on-device transposes of x.
  - scores^T = K^T.T @ Q^T per 128-key-chunk x 512-query-tile, two heads packed
    into disjoint PE row-groups (contraction is only dk=64).
  - softmax without max-subtraction (scores are O(1)); rowsum folded into the
    PV matmul via an augmented V [keys, 64+1] whose last column is ones.
  - exp is evaluated only on the causal part of diagonal chunks; the rest of
    the P tile is memset to 0, and only the 128-wide diagonal strip is masked.
  - reciprocals of the softmax denominators are batched into one DVE op per
    batch; the row broadcast runs on the otherwise-idle GpSimd engine.
  - ctx re-sharded token-parallel with one AllToAll per batch element; the
    batch-0 AllToAll overlaps batch-1 attention. Out-projection runs with full
    W_o on each core for its 2x256 tokens.
  - bf16 matmul inputs everywhere (2.4 GHz PE stream rate); PSUM accumulation
    and softmax normalization stay fp32.

kernel(**inputs) takes the full unsharded inputs and returns the full output.
"""

import numpy as np
import ml_dtypes

import concourse.bass as bass
import concourse.mybir as mybir
import concourse.tile as tile
from concourse import bacc
from concourse.bass import ts
from concourse.bass_utils import run_bass_kernel_spmd

B, S, D = 2, 2048, 1024
H, DK = 16, 64
NCORE = 8
T = B * S          # 4096 tokens
TT = 512           # token tile (projections, q-tiles)
NT = T // TT       # 8
KC = 128           # key chunk
NJ = S // TT       # 4 q-tiles per batch
GG = 256           # a2a token group (per dst core, per batch)
SCALE = 1.0 / np.sqrt(DK)

f32 = mybir.dt.float32
bf16 = mybir.dt.bfloat16
EXP = mybir.ActivationFunctionType.Exp
MULT = mybir.AluOpType.mult
npbf = ml_dtypes.bfloat16


def build_program():
    nc = bacc.Bacc("TRN2", target_bir_lowering=False, debug=False,
                   num_devices=NCORE)

    xT_d = nc.dram_tensor("xT", [128, 8, T], bf16, kind="ExternalInput").ap()
    wT_d = nc.dram_tensor("wT", [128, 8, 3, 128], bf16, kind="ExternalInput").ap()
    woT_d = nc.dram_tensor("woT", [8, 128, 8, 128], bf16, kind="ExternalInput").ap()
    bqkv_d = nc.dram_tensor("bqkv", [128, 3], f32, kind="ExternalInput").ap()
    bo_d = nc.dram_tensor("bo", [128, 8], f32, kind="ExternalInput").ap()
    trimask_d = nc.dram_tensor("trimask", [128, 128], bf16, kind="ExternalInput").ap()
    onescol_d = nc.dram_tensor("onescol", [128, S // KC], bf16, kind="ExternalInput").ap()
    ident_d = nc.dram_tensor("ident", [128, 128], bf16, kind="ExternalInput").ap()
    # out^T for this core's tokens: [od_tile, p, batch, 256]
    outT_d = nc.dram_tensor("outT", [8, 128, B, GG], f32, kind="ExternalOutput").ap()

    with tile.TileContext(nc) as tc:
        with (
            tc.tile_pool(name="const", bufs=1) as constp,
            tc.tile_pool(name="wostream", bufs=3) as wop,
            tc.tile_pool(name="xstream", bufs=2) as xp,
            tc.tile_pool(name="qkv", bufs=1) as qkvp,
            tc.tile_pool(name="vaug", bufs=1) as vaugp,
            tc.tile_pool(name="ptile", bufs=4) as pp,
            tc.tile_pool(name="post", bufs=1) as postp,
            tc.tile_pool(name="cxn", bufs=4) as cxnp,
            tc.tile_pool(name="cxhold", bufs=12) as cxp,
            tc.tile_pool(name="outsb", bufs=2) as outp,
            tc.tile_pool(name="ps_s", bufs=2, space="PSUM") as ps_s,
            tc.tile_pool(name="ps_ctx", bufs=1, space="PSUM") as ps_ctx,
            tc.tile_pool(name="ps_misc", bufs=2, space="PSUM") as ps_misc,
            tc.tile_pool(name="dram", bufs=1, space="DRAM") as dramp,
        ):
            # ---- constants ----
            wT = constp.tile([128, 8, 3, 128], bf16, tag="wT")
            nc.sync.dma_start(wT[:], wT_d)
            bqkv = constp.tile([128, 3], f32, tag="bqkv")
            nc.sync.dma_start(bqkv[:], bqkv_d)
            bo_sb = constp.tile([128, 8], f32, tag="bo")
            nc.sync.dma_start(bo_sb[:], bo_d)
            trimask = constp.tile([128, 128], bf16, tag="trimask")
            nc.sync.dma_start(trimask[:], trimask_d)
            ident = constp.tile([128, 128], bf16, tag="ident")
            nc.sync.dma_start(ident[:], ident_d)

            # ---- stage B: QKV projections (transposed activations) ----
            qkv = [qkvp.tile([128, T], bf16, tag=f"qkv{j}", name=f"qkv{j}")
                   for j in range(3)]
            for t in range(NT):
                xt = xp.tile([128, 8, TT], bf16, tag="xt")
                nc.sync.dma_start(xt[:], xT_d[:, :, ts(t, TT)])
                for j in range(3):
                    ps = ps_misc.tile([128, TT], f32, tag="mm")
                    for o in range(8):
                        nc.tensor.matmul(ps[:], wT[:, o, j, :], xt[:, o, :],
                                         start=(o == 0), stop=(o == 7))
                    nc.vector.tensor_scalar_add(qkv[j][:, ts(t, TT)], ps[:],
                                                bqkv[:, j:j + 1])

            qT_sb, kT_sb, vT_sb = qkv

            # ---- stage C: V^T -> natural-layout V_aug [keys, 65] per (b, head) ----
            vaug = [[vaugp.tile([128, S // KC, DK + 1], bf16,
                                tag=f"vaug{b}{h}", name=f"vaug{b}{h}")
                     for h in range(2)] for b in range(B)]
            for b in range(B):
                for h in range(2):
                    nc.sync.dma_start(vaug[b][h][:, :, DK:DK + 1],
                                      onescol_d[:, :, None])
                for kt in range(S // KC):
                    ps_t = ps_misc.tile([128, TT], bf16, tag="mm")
                    nc.tensor.transpose(ps_t[:, 0:128],
                                        vT_sb[:, b * S + kt * KC:b * S + (kt + 1) * KC],
                                        ident[:])
                    for h in range(2):
                        nc.vector.tensor_copy(vaug[b][h][:, kt, 0:DK],
                                              ps_t[:, DK * h:DK * h + DK])

            # ---- per-batch A2A buffers ----
            a2a_in = [dramp.tile([NCORE, 128, GG], bf16, tag=f"a2a_in{b}",
                                 name=f"a2a_in{b}") for b in range(B)]
            a2a_out = [dramp.tile([NCORE, 128, GG], bf16, tag=f"a2a_out{b}",
                                  name=f"a2a_out{b}") for b in range(B)]

            # rowsum rows collected per batch (as [128, 4] column chunks so a
            # single cheap reciprocal covers all 8 rows) -> recip -> scatter back
            rs_gather = [postp.tile([128, NJ * 2 * 4], f32, tag=f"rs{b}", name=f"rs{b}")
                         for b in range(B)]
            recip_gather = [postp.tile([128, NJ * 2 * 4], f32, tag=f"rcp{b}",
                                       name=f"rcp{b}") for b in range(B)]
            # ctx psum tiles live until the batch post-pass reads them: one per
            # (j, h) would blow PSUM, so copy ctx rows out per q-tile (cx) and
            # only keep the rowsum row in rs_all.
            cx_tiles = {}

            def attention_qtile(b, j):
                nk = 4 * (j + 1)
                qs = b * S + j * TT
                pc = [ps_ctx.tile([DK + 1, TT], f32, tag=f"c{h}", name=f"pc{h}")
                      for h in range(2)]

                def emit_pv(p_tile, m):
                    for h in range(2):
                        nc.tensor.matmul(
                            pc[h][:], vaug[b][h][:, m, :],
                            p_tile[:, TT * h:TT * (h + 1)],
                            start=(m == 0), stop=(m == nk - 1),
                            skip_group_check=True)

                prev = None
                for m in range(nk):
                    ks = b * S + m * KC
                    ps = ps_s.tile([128, 2 * TT], f32, tag="s")
                    nc.tensor.matmul(ps[:, 0:TT], kT_sb[0:DK, ks:ks + KC],
                                     qT_sb[0:DK, qs:qs + TT],
                                     start=True, stop=True, tile_position=(0, 0))
                    nc.tensor.matmul(ps[:, TT:], kT_sb[DK:128, ks:ks + KC],
                                     qT_sb[DK:128, qs:qs + TT],
                                     start=True, stop=True, tile_position=(64, 0))
                    p = pp.tile([128, 2 * TT], bf16, tag="p")
                    r = m - 4 * j
                    if r >= 0:
                        # diagonal chunk: exp only the causal columns, zero the
                        # rest, tri-mask the 128-wide diagonal strip
                        if r > 0:
                            nc.vector.memset(
                                p[:].rearrange("k (h q) -> k h q", h=2)[:, :, 0:KC * r],
                                0.0)
                        nc.scalar.activation(
                            p[:].rearrange("k (h q) -> k h q", h=2)[:, :, KC * r:],
                            ps[:].rearrange("k (h q) -> k h q", h=2)[:, :, KC * r:],
                            EXP, scale=float(SCALE))
                        nc.vector.tensor_tensor(
                            p[:].rearrange("k (h q) -> k h q", h=2)[:, :, KC * r:KC * (r + 1)],
                            p[:].rearrange("k (h q) -> k h q", h=2)[:, :, KC * r:KC * (r + 1)],
                            trimask[:, None, :].to_broadcast([128, 2, 128]), MULT)
                    else:
                        nc.scalar.activation(p[:], ps[:], EXP, scale=float(SCALE))
                    if prev is not None:
                        emit_pv(*prev)
                    prev = (p, m)
                emit_pv(*prev)

                idx2 = j * 2
                for h in range(2):
                    idx = idx2 + h
                    rtmp = cxnp.tile([1, TT], f32, tag="rtmp")
                    nc.vector.tensor_copy(rtmp[:], pc[h][DK:DK + 1, :])
                    nc.sync.dma_start(rs_gather[b][:, 4 * idx:4 * idx + 4],
                                      rtmp[:])
                    cx = cxp.tile([DK, TT], f32, tag="cx", name=f"cx{b}{j}{h}")
                    nc.vector.tensor_copy(cx[:], pc[h][0:DK, :])
                    cx_tiles[(b, j, h)] = cx

            def postpass(b):
                with nc.allow_low_precision(reason="softmax denominator"):
                    nc.vector.reciprocal(recip_gather[b][:], rs_gather[b][:])
                for j in range(NJ):
                    for h in range(2):
                        idx = j * 2 + h
                        rrow = cxnp.tile([1, TT], f32, tag="rrow")
                        nc.sync.dma_start(
                            rrow[:], recip_gather[b][:, 4 * idx:4 * idx + 4])
                        bcast = cxnp.tile([DK, TT], f32, tag="bcast")
                        nc.gpsimd.partition_broadcast(
                            bcast[:], rrow[:], channels=DK)
                        cxn = cxnp.tile([DK, TT], bf16, tag="cxn")
                        nc.vector.tensor_tensor(cxn[:], cx_tiles[(b, j, h)][:],
                                                bcast[:], MULT)
                        for g in range(2):   # 256-token groups -> dst cores 2j+g
                            nc.sync.dma_start(
                                a2a_in[b][2 * j + g, DK * h:DK * (h + 1), :],
                                cxn[:, GG * g:GG * (g + 1)])

            def do_a2a(b):
                nc.gpsimd.collective_compute(
                    "AllToAll", mybir.AluOpType.bypass,
                    replica_groups=[list(range(NCORE))],
                    ins=[a2a_in[b][:].opt()], outs=[a2a_out[b][:].opt()])

            def outproj(b):
                ctx_sb = constp.tile([128, 8, GG], bf16, tag=f"ctx{b}",
                                     name=f"ctx{b}")
                nc.sync.dma_start(ctx_sb[:], a2a_out[b][:].rearrange("r p t -> p r t"))
                for o in range(8):
                    wo_t = wop.tile([128, 8, 128], bf16, tag="wo")
                    nc.sync.dma_start(wo_t[:], woT_d[o])
                    ps = ps_misc.tile([128, TT], f32, tag="mm")
                    for d in range(8):
                        nc.tensor.matmul(ps[:, 0:GG], wo_t[:, d, :], ctx_sb[:, d, :],
                                         start=(d == 0), stop=(d == 7))
                    ot = outp.tile([128, GG], f32, tag="ot")
                    nc.vector.tensor_scalar_add(ot[:], ps[:, 0:GG], bo_sb[:, o:o + 1])
                    nc.sync.dma_start(outT_d[o, :, b, :], ot[:])

            # ---- schedule ----
            for j in range(NJ):
                attention_qtile(0, j)
            attention_qtile(1, 0)
            attention_qtile(1, 1)
            postpass(0)
            do_a2a(0)
            attention_qtile(1, 2)
            attention_qtile(1, 3)
            postpass(1)
            do_a2a(1)
            outproj(0)   # a2a(0) finished long ago; overlaps a2a(1)
            outproj(1)

    nc.compile()
    return nc


def make_in_maps(x, Wq, bq, Wk, bk, Wv, bv, Wo, bo):
    x = np.asarray(x, np.float32)
    xT = np.ascontiguousarray(x.reshape(T, D).T)                  # [D, T]
    xT_t = np.ascontiguousarray(
        xT.reshape(8, 128, T).transpose(1, 0, 2)).astype(npbf)

    woT = np.ascontiguousarray(
        np.asarray(Wo, np.float32).T.reshape(8, 128, 8, 128)
        .transpose(2, 1, 0, 3)).astype(npbf)

    trimask = (np.arange(128)[:, None] <= np.arange(128)[None, :]).astype(npbf)
    onescol = np.ones((128, S // KC), npbf)
    ident = np.eye(128, dtype=npbf)
    bo_t = np.ascontiguousarray(np.asarray(bo, np.float32).reshape(8, 128).T)

    in_maps = []
    for c in range(NCORE):
        sl = slice(128 * c, 128 * (c + 1))
        wT_c = np.stack(
            [np.ascontiguousarray(
                np.asarray(W, np.float32)[sl, :].T.reshape(8, 128, 128)
                .transpose(1, 0, 2))
             for W in (Wq, Wk, Wv)], axis=2)                       # [128, 8, 3, 128]
        bqkv_c = np.stack([np.asarray(b_, np.float32)[sl]
                           for b_ in (bq, bk, bv)], axis=1)        # [128, 3]
        in_maps.append({
            "xT": xT_t,
            "wT": np.ascontiguousarray(wT_c).astype(npbf),
            "woT": woT,
            "bqkv": np.ascontiguousarray(bqkv_c),
            "bo": bo_t,
            "trimask": trimask,
            "onescol": onescol,
            "ident": ident,
        })
    return in_maps


def assemble_output(results):
    # results[c]["outT"]: [8, 128, B, 256] = out^T[od, (b, 256c..256c+256)]
    outT = np.empty((D, B, S), np.float32)
    for c in range(NCORE):
        outT[:, :, GG * c:GG * (c + 1)] = results[c]["outT"].reshape(D, B, GG)
    return np.ascontiguousarray(outT.transpose(1, 2, 0))


_PROGRAM = None


def get_program():
    global _PROGRAM
    if _PROGRAM is None:
        _PROGRAM = build_program()
    return _PROGRAM


def run(in_maps, **kwargs):
    nc = get_program()
    return run_bass_kernel_spmd(nc, in_maps, core_ids=list(range(NCORE)), **kwargs)


def kernel(x, Wq, bq, Wk, bk, Wv, bv, Wo, bo):
    in_maps = make_in_maps(x, Wq, bq, Wk, bk, Wv, bv, Wo, bo)
    res = run(in_maps)
    return assemble_output(res.results)


if __name__ == "__main__":
    rng = np.random.default_rng(0)
    x = rng.standard_normal((B, S, D), dtype=np.float32)
    mk = lambda *s: ((rng.random(s).astype(np.float32)) - 0.5) / 16
    out = kernel(x, mk(D, D), mk(D), mk(D, D), mk(D), mk(D, D), mk(D),
                 mk(D, D), mk(D))
    print(out.shape, out.dtype, np.abs(out).mean())
